# revision 1
# baseline (speedup 1.0000x reference)
"""Chunked sigmoid MHA on 8 Trainium2 NeuronCores (Bass/Tile).

Problem: out = (sigmoid(scale * (x_q Wq^T)(x_k Wk^T)^T) @ (x_v Wv^T)) @ Wo^T
with B=2, L=S=2048, E=1024, H=16, D=64.

Sharding: (batch, head-group) — core c handles batch b=c//4 and heads
[4g, 4g+4) with g=c%4.  Each core computes its 4 heads' Q/K/V projections
(column slices of Wq/Wk/Wv), full sigmoid attention for those heads, and a
partial output projection (row slice of Wo^T); the host sums the 4 partial
outputs per batch.

Device layouts (per core):
  xqT/xkT/xvT [E=1024, L=2048]   host-transposed activations
  wq/wk/wv    [E=1024, 256]      (Wq[g*256:(g+1)*256, :]).T
  wo          [256, E=1024]      (Wo[:, g*256:(g+1)*256]).T
  out         [L=2048, E=1024]   partial output (natural layout)

All matmuls run as float32r (TF32-like single-pass mode, full PE rate for
moving dim >= 256).  Scores matmuls are row-tiled (K=64: two heads packed
in array rows 0-63 / 64-127); attention-output matmuls are col-tiled
(M=64: two heads packed in array cols 0-63 / 64-127 -> PSUM partition
halves).
"""

import ml_dtypes
import numpy as np

import concourse.bass as bass
import concourse.mybir as mybir
import concourse.tile as tile
from concourse import bass_utils
from concourse.vector_clock import ScopedClock

F32 = mybir.dt.float32
F32R = mybir.dt.float32r
BF16 = mybir.dt.bfloat16
AF = mybir.ActivationFunctionType

E = 1024          # embed dim
L = 2048          # sequence length (queries == keys)
DH = 256          # per-core projection dim (4 heads x 64)
EC = E // 128     # 8 E-chunks of 128
LC = L // 512     # 4 L-chunks of 512
ST = L // 128     # 16 S-tiles of 128
SCALE = 64 ** -0.5  # 0.125, applied inside the sigmoid activation

N_CORES = 8


class SplitDrainTileContext(tile.TileContext):
    """This walrus build rejects >1 sync wait on the SP CTRL (Drain)
    instruction, and Tile's end-of-kernel drain waits on every used proc.
    Split the waits across a chain of single-wait drains."""

    DRAIN_WAIT_CAP = 1

    def _drain_and_barrier(self, tick_clock, wait_clock):
        nc = self.nc
        drain_inst = nc.sync.drain()
        wait_clock.add_sem_waits(
            drain_inst.ins, ScopedClock({None: tick_clock.global_clock})
        )
        si = drain_inst.ins.sync_info
        waits = list(si.on_wait) if si is not None else []
        if len(waits) > self.DRAIN_WAIT_CAP:
            si.on_wait = waits[: self.DRAIN_WAIT_CAP]
            for i in range(self.DRAIN_WAIT_CAP, len(waits), self.DRAIN_WAIT_CAP):
                extra = nc.sync.drain()
                esi = extra.ins.sync_info
                if esi is None:
                    esi = mybir.SyncInfo(on_wait=[], on_update=[])
                esi.on_wait = waits[i : i + self.DRAIN_WAIT_CAP]
                extra.ins.sync_info = esi
        nc.all_engine_barrier()
        assert self.sems is not None
        popped = nc._tile_sem_poison_stack.pop()
        assert popped is self._sem_poison
        nc.clear_and_free_semaphores(list(self.sems.allocated().values()))
        nc.all_engine_barrier()


def build_nc() -> bass.Bass:
    nc = bass.Bass("TRN2", target_bir_lowering=False, debug=False)

    xq = nc.dram_tensor("xqT", [E, L], BF16, kind="ExternalInput").ap()
    xk = nc.dram_tensor("xkT", [E, L], BF16, kind="ExternalInput").ap()
    xv = nc.dram_tensor("xvT", [E, L], BF16, kind="ExternalInput").ap()
    wq = nc.dram_tensor("wq", [E, DH], BF16, kind="ExternalInput").ap()
    wk = nc.dram_tensor("wk", [E, DH], BF16, kind="ExternalInput").ap()
    wv = nc.dram_tensor("wv", [E, DH], BF16, kind="ExternalInput").ap()
    wo = nc.dram_tensor("wo", [DH, E], BF16, kind="ExternalInput").ap()
    out = nc.dram_tensor("out", [L, E], F32, kind="ExternalOutput").ap()

    with SplitDrainTileContext(nc) as tc:
        body(tc, xq, xk, xv, wq, wk, wv, wo, out)
    _split_waits(nc)
    return nc


def _split_waits(nc, cap=1):
    """This walrus build rejects instructions carrying more than one sync
    wait.  Hoist excess waits onto same-engine NoOps inserted immediately
    before the instruction (engine program order enforces them first)."""
    ctr = 0
    for f in nc.m.functions:
        for bb in f.blocks:
            new = []
            for inst in bb.instructions:
                si = inst.sync_info
                waits = list(si.on_wait) if si is not None else []
                if len(waits) > cap:
                    for i in range(cap, len(waits), cap):
                        ctr += 1
                        nop = mybir.InstNoOp(name=f"I-waitnop-{ctr}")
                        nop.engine = inst.engine
                        nop.sync_info = mybir.SyncInfo(
                            on_wait=waits[i : i + cap], on_update=[]
                        )
                        nc.register_instruction(nop)
                        new.append(nop)
                    si.on_wait = waits[:cap]
                new.append(inst)
            bb.instructions = new
    return ctr


def body(tc, xq, xk, xv, wq, wk, wv, wo, out):
    nc = tc.nc

    # ---- persistent SBUF tensors -------------------------------------
    persist = tc.alloc_tile_pool(name="persist", bufs=1)

    def ptile(name, shape):
        return persist.tile(shape, BF16, tag=name, name=name)

    # weights, E-chunk-major: w*_sb[:, e*256+m] = w*T[e*128+p, m]
    wq_sb = ptile("wq_sb", [128, EC * DH])
    wk_sb = ptile("wk_sb", [128, EC * DH])
    wv_sb = ptile("wv_sb", [128, EC * DH])
    # wo, m-chunk-major: wo_sb[:, m*1024+e] = wo[m*128+p, e]
    wo_sb = ptile("wo_sb", [128, 2 * E])
    # projected tensors: qT/kT [dh, L] stored Mt-major; v natural [S, dh]
    # stored St-major; oT [dh, L] stored m-chunk-major
    qT_sb = ptile("qT_sb", [128, 2 * L])
    kT_sb = ptile("kT_sb", [128, 2 * L])
    v_sb = persist.tile([128, ST * DH], BF16, tag="v_sb", name="v_sb")
    oT_sb = ptile("oT_sb", [128, 2 * L])

    sc_pool = tc.alloc_tile_pool(name="sc", bufs=8)
    ou_pool = tc.alloc_tile_pool(name="ou", bufs=3)
    xin = tc.alloc_tile_pool(name="xin", bufs=24)
    xin_s = tc.alloc_tile_pool(name="xin_s", bufs=48)
    ps_proj = tc.alloc_tile_pool(name="ps_proj", bufs=2, space="PSUM")
    ps_sc = tc.alloc_tile_pool(name="ps_sc", bufs=2, space="PSUM")
    ps_o = tc.alloc_tile_pool(name="ps_o", bufs=2, space="PSUM")

    def emit_late_weights():
        for m in range(2):
            nc.gpsimd.dma_start(wo_sb[:, m * E : (m + 1) * E], wo[m * 128 : (m + 1) * 128, :])

    # ---- emission helpers --------------------------------------------
    # x tiles span TWO L-chunks ([128, 1024] bf16): half the dma_start
    # count (each issue costs ~0.6us on the issuing engine).  Input DMAs
    # alternate between sync and gpsimd queues to parallelize issue.
    xtiles = {}

    def dma_engines():
        return (nc.sync, nc.gpsimd)

    def dma_startup():
        """Startup-critical DMAs on three parallel engine paths: sync
        carries chunk0 k/q x tiles, scalar carries the weights + chunk0 v
        tiles, gpsimd (SWDGE) carries chunk1.  Per-engine HWDGE queue
        bandwidth (~60-100GB/s) is the startup limiter, not aggregate."""
        for e in range(EC):
            for nm, srcp in (("k", xk), ("q", xq)):
                xt = xin_s.tile([128, 512], BF16, tag="xin_s", name=f"x{nm}c0_{e}")
                nc.sync.dma_start(xt[:], srcp[e * 128 : (e + 1) * 128, 0:512])
                xtiles[(nm, e, "c", 0)] = xt
            nc.scalar.dma_start(wk_sb[:, e * DH : (e + 1) * DH], wk[e * 128 : (e + 1) * 128, :])
            nc.scalar.dma_start(wq_sb[:, e * DH : (e + 1) * DH], wq[e * 128 : (e + 1) * 128, :])
        for e in range(EC):
            nc.scalar.dma_start(wv_sb[:, e * DH : (e + 1) * DH], wv[e * 128 : (e + 1) * 128, :])
            xt = xin_s.tile([128, 512], BF16, tag="xin_s", name=f"xvc0_{e}")
            nc.scalar.dma_start(xt[:], xv[e * 128 : (e + 1) * 128, 0:512])
            xtiles[("v", e, "c", 0)] = xt
        for e in range(EC):
            for nm, srcp in (("k", xk), ("q", xq), ("v", xv)):
                xt = xin_s.tile([128, 512], BF16, tag="xin_s", name=f"x{nm}c1_{e}")
                nc.gpsimd.dma_start(xt[:], srcp[e * 128 : (e + 1) * 128, 512:1024])
                xtiles[(nm, e, "c", 1)] = xt

    def dma_half(h):
        hs = slice(h * 1024, (h + 1) * 1024)
        engs = dma_engines()
        for e in range(EC):
            for idx, (nm, srcp) in enumerate((("k", xk), ("q", xq))):
                xt = xin.tile([128, 1024], BF16, tag="xin", name=f"x{nm}{h}_{e}")
                engs[(e + idx) % 2].dma_start(xt[:], srcp[e * 128 : (e + 1) * 128, hs])
                xtiles[(nm, e, h)] = xt

    def dma_half_v(h):
        hs = slice(h * 1024, (h + 1) * 1024)
        engs = dma_engines()
        for e in range(EC):
            xt = xin.tile([128, 1024], BF16, tag="xin", name=f"xv{h}_{e}")
            engs[e % 2].dma_start(xt[:], xv[e * 128 : (e + 1) * 128, hs])
            xtiles[("v", e, h)] = xt

    def emit_proj_chunk(c, first=False):
        """Yield closures emitting one L-chunk of the projection matmuls
        (x tiles must already be prefetched via dma_half*)."""
        h, p = c // 2, c % 2

        def kq_mms(nm, wsb, dst, e, acc):
            for mt in range(2):
                nc.tensor.matmul(
                    acc[mt][:],
                    lhsT=wsb[:, e * DH + mt * 128 : e * DH + (mt + 1) * 128],
                    rhs=(xtiles[(nm, e, "c", c)][:]
                         if c < 2 else
                         xtiles[(nm, e, h)][:, p * 512 : (p + 1) * 512]),
                    start=(e == 0),
                    stop=(e == EC - 1),
                )
            if e == EC - 1:
                for mt in range(2):
                    nc.vector.tensor_copy(
                        dst[:, mt * L + c * 512 : mt * L + (c + 1) * 512], acc[mt][:]
                    )

        def v_mms(st4, eh, box):
            st = c * 4 + st4
            if eh == 0:
                box["vacc"] = ps_proj.tile([128, DH], F32, tag="ps_proj", name=f"vacc{st}")
            for e in range(eh * 4, eh * 4 + 4):
                nc.tensor.matmul(
                    box["vacc"][:],
                    lhsT=(xtiles[("v", e, "c", c)][:, st4 * 128 : (st4 + 1) * 128]
                          if c < 2 else
                          xtiles[("v", e, h)][:, p * 512 + st4 * 128 : p * 512 + (st4 + 1) * 128]),
                    rhs=wv_sb[:, e * DH : (e + 1) * DH],
                    start=(e == 0),
                    stop=(e == EC - 1),
                )
            if eh == 1:
                nc.vector.tensor_copy(v_sb[:, st * DH : (st + 1) * DH], box["vacc"][:])

        if first:
            yield emit_late_weights
        for nm, wsb, dst in (("k", wk_sb, kT_sb), ("q", wq_sb, qT_sb)):
            acc = [
                ps_proj.tile([128, 512], F32, tag="ps_proj", name=f"{nm}{c}_{mt}")
                for mt in range(2)
            ]
            for e in range(EC):
                yield lambda nm=nm, wsb=wsb, dst=dst, e=e, acc=acc: kq_mms(nm, wsb, dst, e, acc)
        for st4 in range(4):
            box = {}
            for eh in range(2):
                yield lambda st4=st4, eh=eh, box=box: v_mms(st4, eh, box)

    def emit_outproj(lc):
        for lt in range(4):
            def unit(lt=lt):
                lg = lc * 512 + lt * 128
                ot = ou_pool.tile([128, E], F32, tag="ou", name=f"ot{lc}_{lt}")
                for ec in range(2):
                    ps = ps_proj.tile(
                        [128, 512], F32, tag="ps_proj", name=f"ops{lc}_{lt}_{ec}"
                    )
                    for m in range(2):
                        nc.tensor.matmul(
                            ps[:],
                            lhsT=oT_sb[:, m * L + lg : m * L + lg + 128],
                            rhs=wo_sb[:, m * E + ec * 512 : m * E + (ec + 1) * 512],
                            start=(m == 0),
                            stop=(m == 1),
                        )
                    nc.vector.tensor_copy(ot[:, ec * 512 : (ec + 1) * 512], ps[:])
                    for q in range(2):
                        col = ec * 512 + q * 256
                        eng = nc.sync if (lt + ec + q) % 2 == 0 else nc.gpsimd
                        eng.dma_start(
                            out[lg : lg + 128, col : col + 256],
                            ot[:, col : col + 256],
                        )
            yield unit

    # ---- main pipeline ------------------------------------------------
    def attention_step(lc, st, o_acc):
        sc_tiles = {}
        for pair in range(2):
            ps = ps_sc.tile([128, 1024], F32, tag="ps_sc", name=f"scps{lc}_{st}_{pair}")
            for sub in range(2):
                nc.tensor.matmul(
                    ps[:, sub * 512 : (sub + 1) * 512],
                    lhsT=kT_sb[
                        sub * 64 : (sub + 1) * 64,
                        pair * L + st * 128 : pair * L + (st + 1) * 128,
                    ],
                    rhs=qT_sb[
                        sub * 64 : (sub + 1) * 64,
                        pair * L + lc * 512 : pair * L + (lc + 1) * 512,
                    ],
                    start=True,
                    stop=True,
                    tile_position=(sub * 64, 0),
                )
            sc = sc_pool.tile([128, 1024], BF16, tag="sc", name=f"sc{lc}_{st}_{pair}")
            nc.scalar.activation(sc[:], ps[:], AF.Sigmoid, scale=SCALE)
            sc_tiles[pair] = sc
        for pair in range(2):
            for sub in range(2):
                h = pair * 2 + sub
                nc.tensor.matmul(
                    o_acc[pair][sub * 64 : (sub + 1) * 64, :],
                    lhsT=v_sb[:, st * DH + h * 64 : st * DH + (h + 1) * 64],
                    rhs=sc_tiles[pair][:, sub * 512 : (sub + 1) * 512],
                    start=(st == 0),
                    stop=(st == ST - 1),
                    tile_position=(0, sub * 64),
                    # Sim's psum-group bookkeeping mis-addresses
                    # partition-offset groups; has_written is per-element
                    # on HW and the two halves are disjoint.
                    skip_group_check=True,
                )

    filler = []          # queue of pending closures

    def pop_filler(n):
        for _ in range(min(n, len(filler))):
            filler.pop(0)()

    def flush_filler():
        while filler:
            filler.pop(0)()

    # Warm the PE (HAM clock gate) with scratch matmuls while the first
    # DMAs are in flight; they have no data dependencies.
    scratch = persist.tile([128, 512], BF16, tag="scratch", name="scratch")
    nc.gpsimd.memset(scratch[:], 0.0)
    wu_ps = ps_sc.tile([128, 1024], F32, tag="ps_sc", name="warmup_ps")
    for i in range(10):
        nc.tensor.matmul(
            wu_ps[:, :512], lhsT=scratch[:, :128], rhs=scratch[:],
            start=(i == 0), stop=(i == 9),
        )

    # prefetch x tiles for chunks 0-1; run chunk 0's k/q projections now,
    # defer its v projection into the filler so the first scores matmuls
    # reach the PE as early as possible
    dma_startup()
    for u in emit_proj_chunk(0, first=True):
        u()

    for lc in range(LC):
        o_acc = [
            ps_o.tile([128, 512], F32, tag="ps_o", name=f"oacc{lc}_{p}")
            for p in range(2)
        ]
        for sg in range(4):
            if lc == 0 and sg == 0:
                dma_half(1)      # prefetch chunks 2-3 x tiles
                dma_half_v(1)
            if lc == 0 and sg < 3:
                filler.extend(emit_proj_chunk(sg + 1))
            for st4 in range(4):
                st = sg * 4 + st4
                attention_step(lc, st, o_acc)
                # filler after: the first steps' scores must reach the PE
                # early (ramp); later steps stall on ACT slots anyway and
                # the scheduler hoists dependencies as needed
                pop_filler(11 if lc == 0 else 1)
            if lc == 0:
                # next sg's attention needs chunk sg+1 complete
                flush_filler()
        for pair in range(2):
            nc.vector.tensor_copy(
                oT_sb[:, pair * L + lc * 512 : pair * L + (lc + 1) * 512],
                o_acc[pair][:],
            )
        filler.extend(emit_outproj(lc))
        if lc == LC - 1:
            flush_filler()

    # release pools in reverse allocation (stack) order
    for pool in (ps_o, ps_sc, ps_proj, xin_s, xin, ou_pool, sc_pool, persist):
        pool.release()


_NC_CACHE = None


def _get_nc():
    global _NC_CACHE
    if _NC_CACHE is None:
        _NC_CACHE = build_nc()
    return _NC_CACHE


def _prep_in_maps(query, key, value, Wq, Wk, Wv, Wo):
    B = query.shape[0]
    xT = {}
    for b in range(B):
        xT[("q", b)] = np.ascontiguousarray(query[b].T).astype(ml_dtypes.bfloat16)
        xT[("k", b)] = np.ascontiguousarray(key[b].T).astype(ml_dtypes.bfloat16)
        xT[("v", b)] = np.ascontiguousarray(value[b].T).astype(ml_dtypes.bfloat16)
    in_maps = []
    for c in range(N_CORES):
        b, g = c // 4, c % 4
        hs = slice(g * DH, (g + 1) * DH)
        in_maps.append(
            {
                "xqT": xT[("q", b)],
                "xkT": xT[("k", b)],
                "xvT": xT[("v", b)],
                "wq": np.ascontiguousarray(Wq[hs, :].T).astype(ml_dtypes.bfloat16),
                "wk": np.ascontiguousarray(Wk[hs, :].T).astype(ml_dtypes.bfloat16),
                "wv": np.ascontiguousarray(Wv[hs, :].T).astype(ml_dtypes.bfloat16),
                "wo": np.ascontiguousarray(Wo[:, hs].T).astype(ml_dtypes.bfloat16),
            }
        )
    return in_maps


LAST_RESULTS = None


def run_sharded(query, key, value, Wq, Wk, Wv, Wo, trace=False, tmpdir=None):
    global LAST_RESULTS
    if trace:
        # Shim the missing antenv.axon_hooks so NTFF tracing works under axon.
        import sys
        import types

        try:
            import antenv.axon_hooks  # noqa: F401
        except ImportError:
            from trn_agent_boot.trn_boot import _ntff_profile_via_ctypes

            _mod = types.ModuleType("antenv.axon_hooks")
            _hook = _ntff_profile_via_ctypes("/opt/axon/libaxon_pjrt.so")
            _mod.get_axon_ntff_profile_hook = lambda: _hook
            sys.modules["antenv.axon_hooks"] = _mod
        bass_utils.upload_artifacts = lambda tmpdir: tmpdir

    nc = _get_nc()
    in_maps = _prep_in_maps(query, key, value, Wq, Wk, Wv, Wo)
    res = bass_utils.run_bass_kernel_spmd(
        nc, in_maps, core_ids=list(range(N_CORES)), trace=trace, tmpdir=tmpdir
    )
    LAST_RESULTS = res
    B = query.shape[0]
    full = np.zeros((B, L, E), dtype=np.float32)
    for c in range(N_CORES):
        full[c // 4] += res.results[c]["out"]
    return full


def kernel(query, key, value, Wq, Wk, Wv, Wo):
    return run_sharded(query, key, value, Wq, Wk, Wv, Wo, trace=False)



# revision 5
# speedup vs baseline: 1.1469x; 1.1469x over previous
"""Chunked sigmoid MHA on 8 Trainium2 NeuronCores (Bass/Tile).

Problem: out = (sigmoid(scale * (x_q Wq^T)(x_k Wk^T)^T) @ (x_v Wv^T)) @ Wo^T
with B=2, L=S=2048, E=1024, H=16, D=64.

Sharding: (batch, head-group) - core c handles batch b=c//4 and heads
[4g, 4g+4) with g=c%4.  Each core computes its 4 heads' Q/K/V projections
(column slices of Wq/Wk/Wv), full sigmoid attention for those heads, and a
partial output projection (row slice of Wo^T); the host sums the 4 partial
outputs per batch.

v2 layout/schedule:
  - Host pre-arranges x inputs chunk-major ([128, 4*4096]: chunk c of 512
    seq positions, within chunk e-major) so each chunk loads with one
    dma_start of 128 x 8KB descriptors (near line rate), and weights as a
    single [128, 8192] tensor (wk|wq|wv|wo, e/m-chunk-major).
  - DMA priority: three issue rings (sync/scalar HWDGE, gpsimd SWDGE)
    ordered by deadline: xk c0 + xq c0 + wk/wq first (k/q proj chunk 0),
    then xv c0/wv, then later k chunks, with q-proj chunks 1-3 last
    (their scores only run at lc>=1).
  - ACT sigmoid table preloaded at t~0 with a dummy activation.
  - Emission pipeline: scores+sigmoid run ~2 pairs ahead of the
    attention-output stream; projection chunks / output projection are
    interleaved as filler to keep the PE busy during sigmoid latency.
  - Output stored as [128, 1024] f32 blocks (one dma_start each, 4KB
    descriptors).
"""

import ml_dtypes
import numpy as np

import concourse.bass as bass
import concourse.mybir as mybir
import concourse.tile as tile
from concourse import bass_utils
from concourse.vector_clock import ScopedClock

F32 = mybir.dt.float32
BF16 = mybir.dt.bfloat16
AF = mybir.ActivationFunctionType

E = 1024          # embed dim
L = 2048          # sequence length (queries == keys)
DH = 256          # per-core projection dim (4 heads x 64)
EC = E // 128     # 8 E-chunks of 128
LC = L // 512     # 4 L-chunks of 512
ST = L // 128     # 16 S-tiles of 128
CW = 8 * 512      # 4096 cols per x chunk tile
SCALE = 64 ** -0.5  # 0.125, applied inside the sigmoid activation

WK_OFF = 0
WQ_OFF = 2048
WV_OFF = 4096
WO_OFF = 6144

N_CORES = 8


class SplitDrainTileContext(tile.TileContext):
    """This walrus build rejects >1 sync wait on the SP CTRL (Drain)
    instruction, and Tile's end-of-kernel drain waits on every used proc.
    Split the waits across a chain of single-wait drains."""

    DRAIN_WAIT_CAP = 1

    def _drain_and_barrier(self, tick_clock, wait_clock):
        nc = self.nc
        drain_inst = nc.sync.drain()
        wait_clock.add_sem_waits(
            drain_inst.ins, ScopedClock({None: tick_clock.global_clock})
        )
        si = drain_inst.ins.sync_info
        waits = list(si.on_wait) if si is not None else []
        if len(waits) > self.DRAIN_WAIT_CAP:
            si.on_wait = waits[: self.DRAIN_WAIT_CAP]
            for i in range(self.DRAIN_WAIT_CAP, len(waits), self.DRAIN_WAIT_CAP):
                extra = nc.sync.drain()
                esi = extra.ins.sync_info
                if esi is None:
                    esi = mybir.SyncInfo(on_wait=[], on_update=[])
                esi.on_wait = waits[i : i + self.DRAIN_WAIT_CAP]
                extra.ins.sync_info = esi
        nc.all_engine_barrier()
        assert self.sems is not None
        popped = nc._tile_sem_poison_stack.pop()
        assert popped is self._sem_poison
        nc.clear_and_free_semaphores(list(self.sems.allocated().values()))
        nc.all_engine_barrier()


def build_nc() -> bass.Bass:
    nc = bass.Bass("TRN2", target_bir_lowering=False, debug=False)

    xq = nc.dram_tensor("xq", [128, 4 * CW], BF16, kind="ExternalInput").ap()
    xk = nc.dram_tensor("xk", [128, 4 * CW], BF16, kind="ExternalInput").ap()
    xv = nc.dram_tensor("xv", [128, 4 * CW], BF16, kind="ExternalInput").ap()
    wall = nc.dram_tensor("wall", [128, 8192], BF16, kind="ExternalInput").ap()
    out = nc.dram_tensor("out", [L, E], F32, kind="ExternalOutput").ap()

    with SplitDrainTileContext(nc) as tc:
        body(tc, xq, xk, xv, wall, out)
    _split_waits(nc)
    return nc


def _split_waits(nc, cap=1):
    """This walrus build rejects instructions carrying more than one sync
    wait.  Hoist excess waits onto same-engine NoOps inserted immediately
    before the instruction (engine program order enforces them first)."""
    ctr = 0
    for f in nc.m.functions:
        for bb in f.blocks:
            new = []
            for inst in bb.instructions:
                si = inst.sync_info
                waits = list(si.on_wait) if si is not None else []
                if len(waits) > cap:
                    for i in range(cap, len(waits), cap):
                        ctr += 1
                        nop = mybir.InstNoOp(name=f"I-waitnop-{ctr}")
                        nop.engine = inst.engine
                        nop.sync_info = mybir.SyncInfo(
                            on_wait=waits[i : i + cap], on_update=[]
                        )
                        nc.register_instruction(nop)
                        new.append(nop)
                    si.on_wait = waits[:cap]
                new.append(inst)
            bb.instructions = new
    return ctr


def body(tc, xq, xk, xv, wall, out):
    nc = tc.nc

    # ---- persistent SBUF tensors -------------------------------------
    persist = tc.alloc_tile_pool(name="persist", bufs=1)

    def ptile(name, shape):
        return persist.tile(shape, BF16, tag=name, name=name)

    w_sb = ptile("w_sb", [128, 8192])          # wk|wq|wv|wo
    qT_sb = ptile("qT_sb", [128, 2 * L])       # [dh-half-major, L]
    kT_sb = ptile("kT_sb", [128, 2 * L])
    v_sb = ptile("v_sb", [128, ST * DH])       # natural [S, dh], St-major
    oT_sb = ptile("oT_sb", [128, 2 * L])       # m(pair)-chunk-major
    x_sb = {}
    for nm in ("k", "q", "v"):
        for c in range(4):
            x_sb[(nm, c)] = ptile(f"x{nm}{c}", [128, CW])
    scratch = ptile("scratch", [128, 512])
    act_warm = ptile("act_warm", [128, 16])

    sc_pool = tc.alloc_tile_pool(name="sc", bufs=10)
    ou_pool = tc.alloc_tile_pool(name="ou", bufs=3)
    ps_sc = tc.alloc_tile_pool(name="ps_sc", bufs=2, space="PSUM")   # scores
    ps_sm = tc.alloc_tile_pool(name="ps_sm", bufs=2, space="PSUM")   # proj/outproj
    ps_o = tc.alloc_tile_pool(name="ps_o", bufs=2, space="PSUM")     # o_acc

    # ---- DMA priority schedule ---------------------------------------
    # Deadlines (PE time): k/q c0 + wk/wq ~ now; xv c0 + wv ~ +8us;
    # xk c1/c2/c3 before lc0 sg1/2/3; xv c1-3 ~2us later than xk c;
    # wo before outproj (~lc0 end); xq c1-3 before lc1/2/3 scores.
    nc.sync.dma_start(x_sb[("k", 0)][:, 0:2048], xk[:, 0:2048])
    nc.sync.dma_start(x_sb[("k", 0)][:, 2048:CW], xk[:, 2048:CW])
    nc.sync.dma_start(x_sb[("k", 1)][:], xk[:, CW : 2 * CW])
    nc.sync.dma_start(x_sb[("k", 2)][:], xk[:, 2 * CW : 3 * CW])
    nc.sync.dma_start(x_sb[("k", 3)][:], xk[:, 3 * CW : 4 * CW])
    nc.sync.dma_start(x_sb[("q", 2)][:], xq[:, 2 * CW : 3 * CW])

    nc.scalar.dma_start(x_sb[("q", 0)][:, 0:2048], xq[:, 0:2048])
    nc.scalar.dma_start(x_sb[("q", 0)][:, 2048:CW], xq[:, 2048:CW])
    nc.scalar.dma_start(x_sb[("v", 0)][:, 0:2048], xv[:, 0:2048])
    nc.scalar.dma_start(x_sb[("v", 0)][:, 2048:CW], xv[:, 2048:CW])
    nc.scalar.dma_start(x_sb[("v", 2)][:], xv[:, 2 * CW : 3 * CW])
    nc.scalar.dma_start(x_sb[("v", 3)][:], xv[:, 3 * CW : 4 * CW])
    nc.scalar.dma_start(x_sb[("q", 1)][:], xq[:, CW : 2 * CW])
    nc.scalar.dma_start(x_sb[("q", 3)][:], xq[:, 3 * CW : 4 * CW])

    nc.gpsimd.dma_start(w_sb[:, WK_OFF : WK_OFF + 2048], wall[:, WK_OFF : WK_OFF + 2048])
    nc.gpsimd.dma_start(w_sb[:, WQ_OFF : WQ_OFF + 2048], wall[:, WQ_OFF : WQ_OFF + 2048])
    nc.gpsimd.dma_start(w_sb[:, WV_OFF : WV_OFF + 2048], wall[:, WV_OFF : WV_OFF + 2048])
    nc.gpsimd.dma_start(x_sb[("v", 1)][:], xv[:, CW : 2 * CW])
    nc.gpsimd.dma_start(w_sb[:, WO_OFF : WO_OFF + 2048], wall[:, WO_OFF : WO_OFF + 2048])

    # ---- ACT table preload + PE warmup --------------------------------
    # The dummy activation forces the sigmoid ACT_TABLE_LOAD (~2.7us) to
    # run during the initial DMA wait instead of before the first real
    # sigmoid; the scratch matmuls warm the PE HAM clock gate.
    nc.gpsimd.memset(scratch[:], 0.0)
    nc.scalar.activation(act_warm[:], scratch[:, 0:16], AF.Sigmoid, scale=SCALE)
    wu_ps = ps_sc.tile([128, 1024], F32, tag="ps_sc", name="warmup_ps")
    for i in range(10):
        nc.tensor.matmul(
            wu_ps[:, :512], lhsT=scratch[:, :128], rhs=scratch[:],
            start=(i == 0), stop=(i == 9),
        )

    # ---- emission units ----------------------------------------------
    def kq_units(nm, c):
        """16 units: (mt, e) with mt outer; each unit = 1 matmul; psum
        [128,512] acc over e, copy to qT/kT at e==7."""
        woff = WK_OFF if nm == "k" else WQ_OFF
        dst = kT_sb if nm == "k" else qT_sb
        xt = x_sb[(nm, c)]
        for mt in range(2):
            box = {}
            for e in range(EC):
                def unit(nm=nm, c=c, mt=mt, e=e, box=box, woff=woff, dst=dst, xt=xt):
                    if e == 0:
                        box["acc"] = ps_sm.tile(
                            [128, 512], F32, tag="ps_sm", name=f"{nm}{c}m{mt}"
                        )
                    nc.tensor.matmul(
                        box["acc"][:],
                        lhsT=w_sb[:, woff + e * DH + mt * 128 : woff + e * DH + (mt + 1) * 128],
                        rhs=xt[:, e * 512 : (e + 1) * 512],
                        start=(e == 0),
                        stop=(e == EC - 1),
                    )
                    if e == EC - 1:
                        nc.vector.tensor_copy(
                            dst[:, mt * L + c * 512 : mt * L + (c + 1) * 512],
                            box["acc"][:],
                        )
                yield unit

    def v_units(c):
        """8 units: (st4, eh); vacc [128,256] acc over e, copy at eh==1."""
        xt = x_sb[("v", c)]
        for st4 in range(4):
            box = {}
            for eh in range(2):
                def unit(c=c, st4=st4, eh=eh, box=box, xt=xt):
                    st = c * 4 + st4
                    if eh == 0:
                        box["acc"] = ps_sm.tile(
                            [128, 256], F32, tag="ps_sm", name=f"v{st}"
                        )
                    for e in range(eh * 4, eh * 4 + 4):
                        nc.tensor.matmul(
                            box["acc"][:],
                            lhsT=xt[:, e * 512 + st4 * 128 : e * 512 + (st4 + 1) * 128],
                            rhs=w_sb[:, WV_OFF + e * DH : WV_OFF + (e + 1) * DH],
                            start=(e == 0),
                            stop=(e == EC - 1),
                        )
                    if eh == 1:
                        nc.vector.tensor_copy(
                            v_sb[:, st * DH : (st + 1) * DH], box["acc"][:]
                        )
                yield unit

    sc_map = {}

    def sc_pair(lc, st, pair):
        ps = ps_sc.tile([128, 1024], F32, tag="ps_sc", name=f"scps{lc}_{st}_{pair}")
        for sub in range(2):
            nc.tensor.matmul(
                ps[:, sub * 512 : (sub + 1) * 512],
                lhsT=kT_sb[
                    sub * 64 : (sub + 1) * 64,
                    pair * L + st * 128 : pair * L + (st + 1) * 128,
                ],
                rhs=qT_sb[
                    sub * 64 : (sub + 1) * 64,
                    pair * L + lc * 512 : pair * L + (lc + 1) * 512,
                ],
                start=True,
                stop=True,
                tile_position=(sub * 64, 0),
            )
        sc = sc_pool.tile([128, 1024], BF16, tag="sc", name=f"sc{lc}_{st}_{pair}")
        nc.scalar.activation(sc[:], ps[:], AF.Sigmoid, scale=SCALE)
        sc_map[(lc, st, pair)] = sc

    o_acc_cur = [None, None]

    def ao_pair(lc, st, pair):
        if st == 0:
            o_acc_cur[pair] = ps_o.tile(
                [128, 512], F32, tag="ps_o", name=f"oacc{lc}_{pair}"
            )
        sc = sc_map.pop((lc, st, pair))
        for sub in range(2):
            h = pair * 2 + sub
            nc.tensor.matmul(
                o_acc_cur[pair][sub * 64 : (sub + 1) * 64, :],
                lhsT=v_sb[:, st * DH + h * 64 : st * DH + (h + 1) * 64],
                rhs=sc[:, sub * 512 : (sub + 1) * 512],
                start=(st == 0),
                stop=(st == ST - 1),
                tile_position=(0, sub * 64),
                # Sim's psum-group bookkeeping mis-addresses
                # partition-offset groups; has_written is per-element
                # on HW and the two halves are disjoint.
                skip_group_check=True,
            )
        if st == ST - 1 and pair == 1:
            for p in range(2):
                nc.vector.tensor_copy(
                    oT_sb[:, p * L + lc * 512 : (p * L + (lc + 1) * 512)],
                    o_acc_cur[p][:],
                )
            filler.extend(outproj_units(lc))

    def outproj_units(lc):
        for lt in range(4):
            def unit(lc=lc, lt=lt):
                lg = lc * 512 + lt * 128
                ot = ou_pool.tile([128, E], F32, tag="ou", name=f"ot{lc}_{lt}")
                ps = ps_sc.tile([128, 1024], F32, tag="ps_sc", name=f"ops{lc}_{lt}")
                for ec in range(2):
                    for m in range(2):
                        nc.tensor.matmul(
                            ps[:, ec * 512 : (ec + 1) * 512],
                            lhsT=oT_sb[:, m * L + lg : m * L + lg + 128],
                            rhs=w_sb[:, WO_OFF + m * E + ec * 512 : WO_OFF + m * E + (ec + 1) * 512],
                            start=(m == 0),
                            stop=(m == 1),
                        )
                nc.vector.tensor_copy(ot[:], ps[:])
                eng = [nc.scalar, nc.gpsimd, nc.sync][(lc * 4 + lt) % 3]
                eng.dma_start(out[lg : lg + 128, :], ot[:])
            yield unit

    filler = []

    def pop_filler(n=1):
        for _ in range(min(n, len(filler))):
            filler.pop(0)()

    def flush_filler():
        while filler:
            filler.pop(0)()

    def interleave(units_a, units_b):
        """Alternate: one a, one b, until both exhausted."""
        a, b = list(units_a), list(units_b)
        while a or b:
            if a:
                a.pop(0)()
            if b:
                b.pop(0)()

    # ---- lc0: bootstrap pipeline -------------------------------------
    for u in kq_units("k", 0):
        u()
    for u in kq_units("q", 0):
        u()
    sc_pair(0, 0, 0)
    sc_pair(0, 0, 1)
    # v c0 interleaved with remaining sc pairs of sg0
    interleave(
        list(v_units(0)),
        [lambda st=st, p=p: sc_pair(0, st, p) for st in (1, 2, 3) for p in (0, 1)],
    )
    # ao sg0 interleaved with k c1 (2 kq units per ao pair)
    ku = list(kq_units("k", 1))
    for st in range(4):
        for p in range(2):
            if ku:
                ku.pop(0)()
            if ku:
                ku.pop(0)()
            ao_pair(0, st, p)

    for sg in (1, 2, 3):
        # scores of sg interleaved with v c(sg)
        interleave(
            list(v_units(sg)),
            [lambda st=st, p=p: sc_pair(0, sg * 4 + st, p) for st in range(4) for p in (0, 1)],
        )
        # ao of sg interleaved with k c(sg+1) (or q c1 for the last sg)
        nxt = list(kq_units("k", sg + 1)) if sg < 3 else list(kq_units("q", 1))
        for st in range(4):
            for p in range(2):
                if nxt:
                    nxt.pop(0)()
                if nxt:
                    nxt.pop(0)()
                ao_pair(0, sg * 4 + st, p)
        for u in nxt:
            u()

    # ---- lc 1..3: steady pipeline ------------------------------------
    LEAD = 2
    stream = [(lc, st, p) for lc in (1, 2, 3) for st in range(16) for p in (0, 1)]
    n = len(stream)
    # queue q-proj chunks 2,3 as filler during lc1/lc2 (deadline: sc of lc2/lc3)
    q_filler = {2: list(kq_units("q", 2)), 3: list(kq_units("q", 3))}

    for i in range(LEAD):
        sc_pair(*stream[i])
    for i in range(n):
        if i + LEAD < n:
            nlc = stream[i + LEAD][0]
            # force q-proj of chunk nlc emitted before its first sc pair
            if stream[i + LEAD][1:] == (0, 0) and nlc in q_filler:
                for u in q_filler.pop(nlc):
                    u()
            sc_pair(*stream[i + LEAD])
        # drip q-proj filler for the next lc
        nxt_q = min((k for k in q_filler), default=None)
        if nxt_q is not None and q_filler[nxt_q]:
            q_filler[nxt_q].pop(0)()
            if not q_filler[nxt_q]:
                del q_filler[nxt_q]
        pop_filler(1)
        ao_pair(*stream[i])
    flush_filler()

    # release pools in reverse allocation (stack) order
    for pool in (ps_o, ps_sm, ps_sc, ou_pool, sc_pool, persist):
        pool.release()


_NC_CACHE = None


def _get_nc():
    global _NC_CACHE
    if _NC_CACHE is None:
        _NC_CACHE = build_nc()
    return _NC_CACHE


def _chunk_major(xT):
    """[E=1024, L=2048] -> [128, 4*4096]: out[p, c*4096 + e*512 + l] =
    xT[e*128+p, c*512+l]."""
    return np.ascontiguousarray(
        xT.reshape(8, 128, 4, 512).transpose(1, 2, 0, 3).reshape(128, 4 * CW)
    )


def _echunk_major(wT, nchunk, width):
    """[nchunk*128, width] -> [128, nchunk*width]."""
    return wT.reshape(nchunk, 128, width).transpose(1, 0, 2).reshape(128, nchunk * width)


def _prep_in_maps(query, key, value, Wq, Wk, Wv, Wo):
    B = query.shape[0]
    bf = ml_dtypes.bfloat16
    xprep = {}
    for b in range(B):
        for nm, src in (("q", query), ("k", key), ("v", value)):
            xprep[(nm, b)] = _chunk_major(
                np.ascontiguousarray(src[b].T).astype(bf)
            )
    in_maps = []
    for c in range(N_CORES):
        b, g = c // 4, c % 4
        hs = slice(g * DH, (g + 1) * DH)
        wparts = [
            _echunk_major(np.ascontiguousarray(Wk[hs, :].T).astype(bf), 8, DH),
            _echunk_major(np.ascontiguousarray(Wq[hs, :].T).astype(bf), 8, DH),
            _echunk_major(np.ascontiguousarray(Wv[hs, :].T).astype(bf), 8, DH),
            _echunk_major(np.ascontiguousarray(Wo[:, hs].T).astype(bf), 2, E),
        ]
        in_maps.append(
            {
                "xq": xprep[("q", b)],
                "xk": xprep[("k", b)],
                "xv": xprep[("v", b)],
                "wall": np.ascontiguousarray(np.concatenate(wparts, axis=1)),
            }
        )
    return in_maps


LAST_RESULTS = None


def run_sharded(query, key, value, Wq, Wk, Wv, Wo, trace=False, tmpdir=None):
    global LAST_RESULTS
    if trace:
        # Shim the missing antenv.axon_hooks so NTFF tracing works under axon.
        import sys
        import types

        try:
            import antenv.axon_hooks  # noqa: F401
        except ImportError:
            from trn_agent_boot.trn_boot import _ntff_profile_via_ctypes

            _mod = types.ModuleType("antenv.axon_hooks")
            _hook = _ntff_profile_via_ctypes("/opt/axon/libaxon_pjrt.so")
            _mod.get_axon_ntff_profile_hook = lambda: _hook
            sys.modules["antenv.axon_hooks"] = _mod
        bass_utils.upload_artifacts = lambda tmpdir: tmpdir

    nc = _get_nc()
    in_maps = _prep_in_maps(query, key, value, Wq, Wk, Wv, Wo)
    res = bass_utils.run_bass_kernel_spmd(
        nc, in_maps, core_ids=list(range(N_CORES)), trace=trace, tmpdir=tmpdir
    )
    LAST_RESULTS = res
    B = query.shape[0]
    full = np.zeros((B, L, E), dtype=np.float32)
    for c in range(N_CORES):
        full[c // 4] += res.results[c]["out"]
    return full


def kernel(query, key, value, Wq, Wk, Wv, Wo):
    return run_sharded(query, key, value, Wq, Wk, Wv, Wo, trace=False)
